# revision 49
# baseline (speedup 1.0000x reference)
"""Multi-head attention (RoPE, interleaved) for Trainium2, 8-core SPMD.

Problem: x[2,2048,1024] @ Wqkv[1024,3072] -> rope(q,k) -> softmax(qk^T/8)v -> @Wout[1024,1024]
Sharding: core c handles batch b=c//4 and heads hs=[4*(c%4) .. +4) (batch x head-group
parallel). Wqkv column-parallel, Wout row-parallel; host sums the 4 partial outputs
per batch.

Device-side design notes:
- All matmuls in fp16 (cast host-side); accumulation and softmax stay fp32.
- All DRAM inputs are host-packed to [128, X] SBUF layout so every DMA moves
  multi-KB contiguous runs per partition (128 descriptors per tensor).
- q,k are produced TRANSPOSED ([d, n]) by the QKV projection with a per-head
  d-permutation folded into the W columns such that each RoPE pair (2t, 2t+1)
  sits at lanes (i, i+16) of the same 32-lane quadrant.  The partition swap is
  then a single DVE stream_shuffle (mask = i^16 per quadrant), and the combine
  is q*cos + swap(q)*sinb with the pair sign folded into sinb rows.
- Scores are computed transposed (S^T[j,i] = k_j . q_i) so no P transpose is
  needed: the softmax denominator comes from a ones-column appended to V
  ([V|1]^T exp(S^T) = [out^T; l]); exp needs no max-subtraction (|S|<=~6).
- The denominator broadcast runs on GPSIMD (partition_broadcast), v-tile
  evictions run on the (QKV-phase-idle) ACT engine, so PE streams matmuls
  nearly gap-free and the HAM clock gate stays warm; dummy matmuls warm it
  up during the initial DMA.  Heads run in order (1,3,0,2) so the final
  head's norm feeds out-proj without a partition-hop DMA.
- Output projection is weight-stationary ([DIM, N] transposed output, the
  host un-transposes); output DMAs alternate between the two DMA queues.
"""

import sys

import numpy as np

F16 = np.float16

B, N, DIM, H, DH = 2, 2048, 1024, 16, 64
ROPE_BASE = 10000.0
NCORES = 8
HPC = 4  # heads per core
KT = DIM // 128  # 8 k-tiles of the input-feature contraction
NCH = N // 512  # 4 token chunks of 512
NJT = N // 128  # 16 key tiles per head
SCALE = DH**-0.5

_prog_cache = {}


def _concourse():
    try:
        import concourse.bass as bass  # noqa: F401
    except ImportError:
        sys.path.insert(0, "/opt/trn_rl_repo")
    import concourse.bass as bass
    import concourse.tile as tile
    from concourse import mybir

    return bass, tile, mybir


def build_program():
    """One SPMD program; per-core behavior differs only via input data."""
    bass, tile, mybir = _concourse()
    f32 = mybir.dt.float32
    f16 = mybir.dt.float16
    Exp = mybir.ActivationFunctionType.Exp

    from concourse import bacc

    nc = bacc.Bacc(None)
    xt_h = nc.dram_tensor("xt", [128, NCH * KT * 512], f16, kind="ExternalInput")
    wqk_h = nc.dram_tensor("wqk", [128, KT * 512], f16, kind="ExternalInput")
    wv_h = nc.dram_tensor("wv", [128, KT * 256], f16, kind="ExternalInput")
    wout_h = nc.dram_tensor("wout", [128, 2 * DIM], f16, kind="ExternalInput")
    cos_h = nc.dram_tensor("cosb", [128, N], f16, kind="ExternalInput")
    sin_h = nc.dram_tensor("sinb", [128, N], f16, kind="ExternalInput")
    # transposed output [DIM, N]; the host un-transposes (free on CPU)
    outp_h = nc.dram_tensor("outp", [DIM, N], f16, kind="ExternalOutput")

    # RoPE pair swap: lanes i <-> i+16 within each 32-lane quadrant.
    SWAP_MASK = [(i + 16) % 32 for i in range(32)]

    with tile.TileContext(nc) as tc:
        with (
            tc.tile_pool(name="consts", bufs=1) as consts,
            tc.tile_pool(name="big", bufs=1) as big,
        ):
            warm = consts.tile([128, 512], f16)
            nc.vector.memset(warm, 0.0)
            v_sb = big.tile([128, NJT, HPC, DH + 1], f16)
            nc.gpsimd.memset(v_sb, 1.0)

            # DMA order = consumption order: wqk + xt0 gate the first matmul,
            # wout isn't needed until the output projection
            wqk_sb = consts.tile([128, KT, 512], f16)
            nc.sync.dma_start(out=wqk_sb, in_=wqk_h[:, :])
            xt_sb = big.tile([128, NCH, KT, 512], f16)
            nc.sync.dma_start(out=xt_sb[:, 0], in_=xt_h[:, 0 : KT * 512])
            cos_sb = consts.tile([128, N], f16)
            nc.sync.dma_start(out=cos_sb, in_=cos_h[:, :])
            sin_sb = consts.tile([128, N], f16)
            nc.sync.dma_start(out=sin_sb, in_=sin_h[:, :])
            nc.sync.dma_start(out=xt_sb[:, 1], in_=xt_h[:, KT * 512 : 2 * KT * 512])
            wv_sb = consts.tile([128, KT, 256], f16)
            nc.sync.dma_start(out=wv_sb, in_=wv_h[:, :])
            for ch in (2, 3):
                nc.sync.dma_start(
                    out=xt_sb[:, ch], in_=xt_h[:, KT * 512 * ch : KT * 512 * (ch + 1)]
                )
            # wout is not needed until the output projection at the very end
            wout_sb = consts.tile([128, 2, DIM], f16)
            nc.sync.dma_start(out=wout_sb, in_=wout_h[:, :])

            qkT = big.tile([128, 4, N], f16)
            ao = big.tile([128, 2, N], f16)  # normalized attn out^T, 2 head-pair tiles

            # ---- HAM warm-up: dummy matmuls while the DMAs stream ----
            with tc.tile_pool(name="warmps", bufs=2, space="PSUM") as wps:
                for w in range(25):
                    dps = wps.tile([128, 512], f32, tag="w")
                    nc.tensor.matmul(
                        dps, warm[:, 0:128], warm, start=True, stop=True
                    )

            # ---- QKV projection + RoPE, per 512-token chunk ----
            with (
                tc.tile_pool(name="ppq", bufs=4, space="PSUM") as ppq,
                tc.tile_pool(name="ppv", bufs=2, space="PSUM") as ppv,
                tc.tile_pool(name="rt", bufs=3) as rt,
            ):
                for ch in range(NCH):
                    sl = slice(512 * ch, 512 * ch + 512)
                    for mt in range(4):
                        ps = ppq.tile([128, 512], f32, tag="ps")
                        for a in range(KT):
                            nc.tensor.matmul(
                                ps,
                                wqk_sb[:, a, 128 * mt : 128 * mt + 128],
                                xt_sb[:, ch, a, :],
                                start=(a == 0),
                                stop=(a == KT - 1),
                            )
                        # RoPE: qkT = ps*cos + shuffle(ps)*sinb, signs in sinb
                        # (shuffle can't cast, so swp stays f32; the sin-mul
                        # runs on the QKV-phase-idle GPSIMD engine)
                        swp = rt.tile([128, 512], f32, tag="swp")
                        nc.vector.stream_shuffle(swp, ps, SWAP_MASK)
                        t2 = rt.tile([128, 512], f16, tag="t2")
                        nc.gpsimd.tensor_mul(t2, swp, sin_sb[:, sl])
                        t1 = rt.tile([128, 512], f16, tag="t1")
                        nc.vector.tensor_mul(t1, ps, cos_sb[:, sl])
                        nc.vector.tensor_add(qkT[:, mt, sl], t1, t2)
                    for tt in range(4):
                        psv = ppv.tile([128, 256], f32, tag="psv")
                        for a in range(KT):
                            nc.tensor.matmul(
                                psv,
                                xt_sb[:, ch, a, 128 * tt : 128 * tt + 128],
                                wv_sb[:, a, :],
                                start=(a == 0),
                                stop=(a == KT - 1),
                            )
                        j = 4 * ch + tt
                        # evict on the (idle-here) ACT engine
                        nc.scalar.copy(
                            v_sb[:, j, :, 0:DH],
                            psv[:, :].rearrange("p (h d) -> p h d", h=HPC),
                        )

            # ---- attention: S^T = k q^T, exp, [V|1]^T P^T accumulation ----
            with (
                tc.tile_pool(name="es", bufs=8) as esp,
                tc.tile_pool(name="pss", bufs=2, space="PSUM") as pss,
                tc.tile_pool(name="psa", bufs=1, space="PSUM") as psa,
                tc.tile_pool(name="nrm", bufs=2) as nrm,
            ):
                # odd heads (partition-hop to ao rows 64-127) first, so the
                # final head's norm chain feeds out-proj without a hop
                for h in (1, 3, 0, 2):
                    tq, r0 = h // 2, 64 * (h % 2)
                    tk = 2 + h // 2
                    qrow = slice(r0, r0 + 64)
                    acc = [
                        psa.tile([128, 512], f32, tag=f"acc{ic}", name=f"acc{h}_{ic}")
                        for ic in range(4)
                    ]
                    avs = [None] * 4

                    def emit_pv(j, estiles, h=h, acc=acc, avs=avs):
                        for half in range(2):
                            for k in range(2):
                                ic = 2 * half + k
                                nc.tensor.matmul(
                                    acc[ic][0:65, :],
                                    v_sb[:, j, h, :],
                                    estiles[half][:, 512 * k : 512 * k + 512],
                                    start=(j == 0),
                                    stop=(j == NJT - 1),
                                )
                                if j == NJT - 1:
                                    # last head: ACT is idle from here on, so
                                    # split the eviction across ACT and DVE to
                                    # shorten the norm chain feeding out-proj
                                    ev = (
                                        nc.scalar.copy
                                        if (h == 2 and ic % 2 == 0)
                                        else nc.vector.tensor_copy
                                    )
                                    av = nrm.tile(
                                        [65, 512], f16, tag=f"av{ic}",
                                        name=f"av{h}_{ic}",
                                    )
                                    ev(av, acc[ic][0:65, :])
                                    lr = nrm.tile(
                                        [1, 512], f32, tag=f"lr{ic}",
                                        name=f"lr{h}_{ic}",
                                    )
                                    ev(lr, acc[ic][64:65, :])
                                    avs[ic] = (av, lr)

                    # PV is software-pipelined one iteration behind QK/exp:
                    # in the PE FIFO the slot between QK(j) and QK(j+1) holds
                    # PV(j-1), whose exp dependency resolved an iteration ago,
                    # so QK(j+1) is never stuck behind PVs waiting on exp(j).
                    prev = None
                    for j in range(NJT):
                        estiles = []
                        for half in range(2):
                            ps = pss.tile([128, 1024], f32, tag="s")
                            for k in range(2):
                                ic = 2 * half + k
                                nc.tensor.matmul(
                                    ps[:, 512 * k : 512 * k + 512],
                                    qkT[qrow, tk, 128 * j : 128 * j + 128],
                                    qkT[qrow, tq, 512 * ic : 512 * ic + 512],
                                    start=True,
                                    stop=True,
                                )
                            es = esp.tile([128, 1024], f16, tag="es")
                            nc.scalar.activation(es[:], ps, Exp, scale=SCALE)
                            estiles.append(es)
                        if prev is not None:
                            emit_pv(j - 1, prev)
                        prev = estiles
                    emit_pv(NJT - 1, prev)
                    # normalize: rcb = bcast(1/l) on GPSIMD, ao = av * rcb
                    ahi = None
                    if r0 == 64:
                        # odd head: normalize at partitions 0-63, then DMA-hop
                        # down to partitions 64-127 of ao
                        ahi = nrm.tile([64, N], f16, tag="ahi", name=f"ahi{h}")
                    for ic, (av, lr) in enumerate(avs):
                        csl = slice(512 * ic, 512 * ic + 512)
                        lb = nrm.tile(
                            [64, 512], f32, tag=f"lb{ic}", name=f"lb{h}_{ic}"
                        )
                        nc.gpsimd.partition_broadcast(lb, lr, 64)
                        rcb = nrm.tile(
                            [64, 512], f32, tag=f"rcb{ic}", name=f"rcb{h}_{ic}"
                        )
                        nc.vector.reciprocal_approx_fast(rcb, lb)
                        tgt = ao[0:64, h // 2, csl] if r0 == 0 else ahi[0:64, csl]
                        nc.vector.tensor_mul(tgt, av[0:64, :], rcb)
                        if r0 == 64:
                            nc.gpsimd.dma_start(
                                out=ao[64:128, h // 2, csl], in_=ahi[0:64, csl]
                            )

                # Bridge the last head's eviction/norm chain (~3us of PE idle
                # before out-proj's PSUM frees) with dummy matmuls: without
                # them the HAM clock gate re-throttles here and the whole
                # output projection runs at 1.2 GHz instead of 2.4.
                for w in range(10):
                    dps = pss.tile([128, 1024], f32, tag="s", name=f"warm2_{w}")
                    nc.tensor.matmul(
                        dps[:, 0:512], warm[:, 0:128], warm, start=True, stop=True
                    )

            # ---- output projection (row-parallel partial) ----
            # Weight-stationary orientation: out^T[n, i] accumulated per
            # 128-wide n-tile; wout is the lhsT so each weight tile serves 8
            # back-to-back matmuls; output DMAs as [DIM, N] (host transposes).
            with (
                tc.tile_pool(name="po", bufs=2, space="PSUM") as pop,
                tc.tile_pool(name="ob", bufs=3) as obp,
            ):
                for nk in range(8):
                    po = pop.tile([128, 4, 512], f32, tag="po")
                    for ct in range(2):
                        for tc4 in range(4):
                            nc.tensor.matmul(
                                po[:, tc4, :],
                                wout_sb[:, ct, 128 * nk : 128 * nk + 128],
                                ao[:, ct, 512 * tc4 : 512 * tc4 + 512],
                                start=(ct == 0),
                                stop=(ct == 1),
                            )
                    ob = obp.tile([128, 4, 512], f16, tag="ob")
                    # quarter-evictions alternating ACT/DVE: po's banks free
                    # ~0.9us after its last matmul instead of ~2.2us, so the
                    # po rotation never bubbles the PE
                    for q in range(4):
                        if (q + nk) % 2 == 0:
                            nc.scalar.copy(ob[:, q, :], po[:, q, :])
                        else:
                            nc.vector.tensor_copy(ob[:, q, :], po[:, q, :])
                    r = slice(128 * nk, 128 * nk + 128)
                    if nk == 7:
                        # last tile: split the DMA across both queues to
                        # shorten the drain tail
                        nc.sync.dma_start(out=outp_h[r, 0:1024], in_=ob[:, 0:2, :])
                        nc.gpsimd.dma_start(
                            out=outp_h[r, 1024:2048], in_=ob[:, 2:4, :]
                        )
                    else:
                        # alternate DMA queues so transfers run in parallel
                        dq = nc.sync if nk % 2 == 0 else nc.gpsimd
                        dq.dma_start(out=outp_h[r, :], in_=ob)
    nc.finalize()
    return nc


# Per-head d-permutation: SBUF row r (0..63) holds head dim DPERM[r].
# Rows 0-15: evens of pairs t=0..15, 16-31: odds of t=0..15,
# 32-47: evens of t=16..31, 48-63: odds of t=16..31.
DPERM = (
    [2 * t for t in range(16)]
    + [2 * t + 1 for t in range(16)]
    + [2 * t for t in range(16, 32)]
    + [2 * t + 1 for t in range(16, 32)]
)
# freq index for row r: t = r%16 + 16*(r//32)
ROW_T = [r % 16 + 16 * (r // 32) for r in range(64)]
# sign of the sin term for row r: -1 for even-output rows (r%32 < 16)
ROW_SIGN = [-1.0 if (r % 32) < 16 else 1.0 for r in range(64)]


def make_core_inputs(x, Wqkv, Wout, c):
    """Host-side shard prep for core c: batch b=c//4, heads [4*(c%4) .. +4)."""
    b = c // 4
    g = c % 4
    hs = [4 * g + i for i in range(HPC)]
    W4 = np.asarray(Wqkv, np.float32).reshape(DIM, 3, H, DH)
    xt = np.asarray(x, np.float32)[b].T  # [DIM, N]

    # xt packed [p][ch][a][n]
    xt_p = xt.reshape(KT, 128, NCH, 512).transpose(1, 2, 0, 3)  # [128, ch, a, n]
    xt_pack = np.ascontiguousarray(xt_p.reshape(128, NCH * KT * 512))

    # wqk columns: tiles mt=0,1 -> q pairs, mt=2,3 -> k pairs; 64 d-permuted
    # cols per head, head A then head B within each tile.
    cols = []
    for qk in (0, 1):
        for pair in (0, 1):
            for hh in (hs[2 * pair], hs[2 * pair + 1]):
                cols.append(W4[:, qk, hh, :][:, DPERM])
    wqk = np.concatenate(cols, axis=1)  # [DIM, 512]
    wqk_pack = np.ascontiguousarray(
        wqk.reshape(KT, 128, 512).transpose(1, 0, 2).reshape(128, KT * 512)
    )

    wv = W4[:, 2, hs, :].reshape(DIM, 256)
    wv_pack = np.ascontiguousarray(
        wv.reshape(KT, 128, 256).transpose(1, 0, 2).reshape(128, KT * 256)
    )

    wout = np.asarray(Wout, np.float32).reshape(H, DH, DIM)[hs].reshape(256, DIM)
    wout_pack = np.ascontiguousarray(
        wout.reshape(2, 128, DIM).transpose(1, 0, 2).reshape(128, 2 * DIM)
    )

    pos = np.arange(N, dtype=np.float64)
    inv = 1.0 / (ROPE_BASE ** (np.arange(0, DH, 2, dtype=np.float64) / DH))  # [32]
    ang = inv[:, None] * pos[None, :]  # [32, N]
    cos_t = np.cos(ang)  # [32 freqs, N]
    sin_t = np.sin(ang)
    rows_t = np.array(ROW_T * 2)  # 128 rows (two 64-row head halves)
    sign = np.array(ROW_SIGN * 2)[:, None]
    cosb = cos_t[rows_t].astype(np.float32)  # [128, N]
    sinb = (sign * sin_t[rows_t]).astype(np.float32)

    return {
        "xt": xt_pack.astype(F16),
        "wqk": wqk_pack.astype(F16),
        "wv": wv_pack.astype(F16),
        "wout": wout_pack.astype(F16),
        "cosb": cosb.astype(F16),
        "sinb": sinb.astype(F16),
    }


def kernel(x, Wqkv, Wout, _trace=False, _tmpdir=None):
    _concourse()
    from concourse.bass_utils import run_bass_kernel_spmd

    if "nc" not in _prog_cache:
        _prog_cache["nc"] = build_program()
    nc = _prog_cache["nc"]
    in_maps = [make_core_inputs(x, Wqkv, Wout, c) for c in range(NCORES)]
    res = run_bass_kernel_spmd(
        nc, in_maps, list(range(NCORES)), trace=_trace, tmpdir=_tmpdir
    )
    out = np.zeros((B, N, DIM), np.float32)
    for c in range(NCORES):
        out[c // 4] += res.results[c]["outp"].astype(np.float32).T
    if _trace:
        return out, res
    return out


# revision 52
# speedup vs baseline: 1.1891x; 1.1891x over previous
"""Multi-head attention (RoPE, interleaved) for Trainium2, 8-core SPMD.

Problem: x[2,2048,1024] @ Wqkv[1024,3072] -> rope(q,k) -> softmax(qk^T/8)v -> @Wout[1024,1024]
Sharding: core c handles batch b=c//4 and heads hs=[4*(c%4) .. +4) (batch x head-group
parallel). Wqkv column-parallel, Wout row-parallel; host sums the 4 partial outputs
per batch.

Device-side design notes:
- All matmuls in fp16 (cast host-side); accumulation and softmax stay fp32.
- All DRAM inputs are host-packed to [128, X] SBUF layout so every DMA moves
  multi-KB contiguous runs per partition (128 descriptors per tensor).
- q,k are produced TRANSPOSED ([d, n]) by the QKV projection with a per-head
  d-permutation folded into the W columns such that each RoPE pair (2t, 2t+1)
  sits at lanes (i, i+16) of the same 32-lane quadrant.  The partition swap is
  then a single DVE stream_shuffle (mask = i^16 per quadrant), and the combine
  is q*cos + swap(q)*sinb with the pair sign folded into sinb rows.
- Scores are computed transposed (S^T[j,i] = k_j . q_i) so no P transpose is
  needed: the softmax denominator comes from a ones-column appended to V
  ([V|1]^T exp(S^T) = [out^T; l]); exp needs no max-subtraction (|S|<=~6).
- The denominator broadcast runs on GPSIMD (partition_broadcast), v-tile
  evictions run on the (QKV-phase-idle) ACT engine, so PE streams matmuls
  nearly gap-free and the HAM clock gate stays warm; dummy matmuls warm it
  up during the initial DMA.  Heads run in order (1,3,0,2) so the final
  head's norm feeds out-proj without a partition-hop DMA.
- Output projection is weight-stationary ([DIM, N] transposed output, the
  host un-transposes); output DMAs alternate between the two DMA queues.
"""

import sys

import numpy as np

F16 = np.float16

B, N, DIM, H, DH = 2, 2048, 1024, 16, 64
ROPE_BASE = 10000.0
NCORES = 8
HPC = 4  # heads per core
KT = DIM // 128  # 8 k-tiles of the input-feature contraction
NCH = N // 512  # 4 token chunks of 512
NJT = N // 128  # 16 key tiles per head
SCALE = DH**-0.5

_prog_cache = {}


def _concourse():
    try:
        import concourse.bass as bass  # noqa: F401
    except ImportError:
        sys.path.insert(0, "/opt/trn_rl_repo")
    import concourse.bass as bass
    import concourse.tile as tile
    from concourse import mybir

    return bass, tile, mybir


def build_program():
    """One SPMD program; per-core behavior differs only via input data."""
    bass, tile, mybir = _concourse()
    f32 = mybir.dt.float32
    f16 = mybir.dt.float16
    Exp = mybir.ActivationFunctionType.Exp

    from concourse import bacc

    nc = bacc.Bacc(None)
    xt_h = nc.dram_tensor("xt", [128, NCH * KT * 512], f16, kind="ExternalInput")
    wqk_h = nc.dram_tensor("wqk", [128, KT * 512], f16, kind="ExternalInput")
    wv_h = nc.dram_tensor("wv", [128, KT * 256], f16, kind="ExternalInput")
    wout_h = nc.dram_tensor("wout", [128, 2 * DIM], f16, kind="ExternalInput")
    cos_h = nc.dram_tensor("cosb", [128, N], f16, kind="ExternalInput")
    sin_h = nc.dram_tensor("sinb", [128, N], f16, kind="ExternalInput")
    # transposed output [DIM, N]; the host un-transposes (free on CPU)
    outp_h = nc.dram_tensor("outp", [DIM, N], f16, kind="ExternalOutput")

    # RoPE pair swap: lanes i <-> i+16 within each 32-lane quadrant.
    SWAP_MASK = [(i + 16) % 32 for i in range(32)]

    with tile.TileContext(nc) as tc:
        with (
            tc.tile_pool(name="consts", bufs=1) as consts,
            tc.tile_pool(name="big", bufs=1) as big,
        ):
            warm = consts.tile([128, 512], f16)
            nc.vector.memset(warm, 0.0)
            v_sb = big.tile([128, NJT, HPC, DH + 1], f16)
            nc.gpsimd.memset(v_sb, 1.0)

            # DMA order = consumption order: wqk + xt0 gate the first matmul,
            # wout isn't needed until the output projection
            wqk_sb = consts.tile([128, KT, 512], f16)
            nc.sync.dma_start(out=wqk_sb, in_=wqk_h[:, :])
            xt_sb = big.tile([128, NCH, KT, 512], f16)
            nc.sync.dma_start(out=xt_sb[:, 0], in_=xt_h[:, 0 : KT * 512])
            cos_sb = consts.tile([128, N], f16)
            nc.sync.dma_start(out=cos_sb, in_=cos_h[:, :])
            sin_sb = consts.tile([128, N], f16)
            nc.sync.dma_start(out=sin_sb, in_=sin_h[:, :])
            nc.sync.dma_start(out=xt_sb[:, 1], in_=xt_h[:, KT * 512 : 2 * KT * 512])
            wv_sb = consts.tile([128, KT, 256], f16)
            nc.sync.dma_start(out=wv_sb, in_=wv_h[:, :])
            for ch in (2, 3):
                nc.sync.dma_start(
                    out=xt_sb[:, ch], in_=xt_h[:, KT * 512 * ch : KT * 512 * (ch + 1)]
                )
            # wout is not needed until the output projection at the very end
            wout_sb = consts.tile([128, 2, DIM], f16)
            nc.sync.dma_start(out=wout_sb, in_=wout_h[:, :])

            qkT = big.tile([128, 4, N], f16)
            ao = big.tile([128, 2, N], f16)  # normalized attn out^T, 2 head-pair tiles

            # ---- HAM warm-up: dummy matmuls while the DMAs stream ----
            with tc.tile_pool(name="warmps", bufs=2, space="PSUM") as wps:
                for w in range(25):
                    dps = wps.tile([128, 512], f32, tag="w")
                    nc.tensor.matmul(
                        dps, warm[:, 0:128], warm, start=True, stop=True
                    )

            # ---- QKV projection + RoPE, per 512-token chunk ----
            with (
                tc.tile_pool(name="ppq", bufs=4, space="PSUM") as ppq,
                tc.tile_pool(name="ppv", bufs=2, space="PSUM") as ppv,
                tc.tile_pool(name="rt", bufs=3) as rt,
            ):
                for ch in range(NCH):
                    sl = slice(512 * ch, 512 * ch + 512)
                    for mt in range(4):
                        ps = ppq.tile([128, 512], f32, tag="ps")
                        for a in range(KT):
                            nc.tensor.matmul(
                                ps,
                                wqk_sb[:, a, 128 * mt : 128 * mt + 128],
                                xt_sb[:, ch, a, :],
                                start=(a == 0),
                                stop=(a == KT - 1),
                            )
                        # RoPE: qkT = ps*cos + shuffle(ps)*sinb, signs in sinb
                        # (shuffle can't cast, so swp stays f32; the sin-mul
                        # runs on the QKV-phase-idle GPSIMD engine)
                        swp = rt.tile([128, 512], f32, tag="swp")
                        nc.vector.stream_shuffle(swp, ps, SWAP_MASK)
                        t2 = rt.tile([128, 512], f16, tag="t2")
                        nc.gpsimd.tensor_mul(t2, swp, sin_sb[:, sl])
                        t1 = rt.tile([128, 512], f16, tag="t1")
                        nc.vector.tensor_mul(t1, ps, cos_sb[:, sl])
                        nc.vector.tensor_add(qkT[:, mt, sl], t1, t2)
                    for tt in range(4):
                        psv = ppv.tile([128, 256], f32, tag="psv")
                        for a in range(KT):
                            nc.tensor.matmul(
                                psv,
                                xt_sb[:, ch, a, 128 * tt : 128 * tt + 128],
                                wv_sb[:, a, :],
                                start=(a == 0),
                                stop=(a == KT - 1),
                            )
                        j = 4 * ch + tt
                        # evict on the (idle-here) ACT engine
                        nc.scalar.copy(
                            v_sb[:, j, :, 0:DH],
                            psv[:, :].rearrange("p (h d) -> p h d", h=HPC),
                        )

            # ---- attention: S^T = k q^T, exp, [V|1]^T P^T accumulation ----
            with (
                tc.tile_pool(name="es", bufs=8) as esp,
                tc.tile_pool(name="pss", bufs=2, space="PSUM") as pss,
                tc.tile_pool(name="psa", bufs=1, space="PSUM") as psa,
                tc.tile_pool(name="nrm", bufs=2) as nrm,
            ):
                # odd heads (partition-hop to ao rows 64-127) first, so the
                # final head's norm chain feeds out-proj without a hop
                for h in (1, 3, 0, 2):
                    tq, r0 = h // 2, 64 * (h % 2)
                    tk = 2 + h // 2
                    qrow = slice(r0, r0 + 64)
                    acc = [
                        psa.tile([128, 512], f32, tag=f"acc{ic}", name=f"acc{h}_{ic}")
                        for ic in range(4)
                    ]
                    avs = [None] * 4

                    def emit_pv(j, estiles, h=h, acc=acc, avs=avs):
                        for half in range(2):
                            for k in range(2):
                                ic = 2 * half + k
                                nc.tensor.matmul(
                                    acc[ic][0:65, :],
                                    v_sb[:, j, h, :],
                                    estiles[half][:, 512 * k : 512 * k + 512],
                                    start=(j == 0),
                                    stop=(j == NJT - 1),
                                )
                                if j == NJT - 1:
                                    # last head: ACT is idle from here on, so
                                    # split the eviction across ACT and DVE to
                                    # shorten the norm chain feeding out-proj
                                    ev = (
                                        nc.scalar.copy
                                        if (h == 2 and ic % 2 == 0)
                                        else nc.vector.tensor_copy
                                    )
                                    av = nrm.tile(
                                        [65, 512], f16, tag=f"av{ic}",
                                        name=f"av{h}_{ic}",
                                    )
                                    ev(av, acc[ic][0:65, :])
                                    lr = nrm.tile(
                                        [1, 512], f32, tag=f"lr{ic}",
                                        name=f"lr{h}_{ic}",
                                    )
                                    ev(lr, acc[ic][64:65, :])
                                    avs[ic] = (av, lr)

                    # PV is software-pipelined one iteration behind QK/exp:
                    # in the PE FIFO the slot between QK(j) and QK(j+1) holds
                    # PV(j-1), whose exp dependency resolved an iteration ago,
                    # so QK(j+1) is never stuck behind PVs waiting on exp(j).
                    prev = None
                    for j in range(NJT):
                        estiles = []
                        for half in range(2):
                            ps = pss.tile([128, 1024], f32, tag="s")
                            for k in range(2):
                                ic = 2 * half + k
                                nc.tensor.matmul(
                                    ps[:, 512 * k : 512 * k + 512],
                                    qkT[qrow, tk, 128 * j : 128 * j + 128],
                                    qkT[qrow, tq, 512 * ic : 512 * ic + 512],
                                    start=True,
                                    stop=True,
                                )
                            es = esp.tile([128, 1024], f16, tag="es")
                            nc.scalar.activation(es[:], ps, Exp, scale=SCALE)
                            estiles.append(es)
                        if prev is not None:
                            emit_pv(j - 1, prev)
                        prev = estiles
                    emit_pv(NJT - 1, prev)
                    # normalize: rcb = bcast(1/l) on GPSIMD, ao = av * rcb
                    ahi = None
                    if r0 == 64:
                        # odd head: normalize at partitions 0-63, then DMA-hop
                        # down to partitions 64-127 of ao
                        ahi = nrm.tile([64, N], f16, tag="ahi", name=f"ahi{h}")
                    for ic, (av, lr) in enumerate(avs):
                        csl = slice(512 * ic, 512 * ic + 512)
                        lb = nrm.tile(
                            [64, 512], f32, tag=f"lb{ic}", name=f"lb{h}_{ic}"
                        )
                        nc.gpsimd.partition_broadcast(lb, lr, 64)
                        rcb = nrm.tile(
                            [64, 512], f32, tag=f"rcb{ic}", name=f"rcb{h}_{ic}"
                        )
                        nc.vector.reciprocal_approx_fast(rcb, lb)
                        tgt = ao[0:64, h // 2, csl] if r0 == 0 else ahi[0:64, csl]
                        nc.vector.tensor_mul(tgt, av[0:64, :], rcb)
                        if r0 == 64:
                            nc.gpsimd.dma_start(
                                out=ao[64:128, h // 2, csl], in_=ahi[0:64, csl]
                            )

                # Bridge the last head's eviction/norm chain (~3us of PE idle
                # before out-proj's PSUM frees) with dummy matmuls: without
                # them the HAM clock gate re-throttles here and the whole
                # output projection runs at 1.2 GHz instead of 2.4.
                for w in range(10):
                    dps = pss.tile([128, 1024], f32, tag="s", name=f"warm2_{w}")
                    nc.tensor.matmul(
                        dps[:, 0:512], warm[:, 0:128], warm, start=True, stop=True
                    )

            # ---- output projection (row-parallel partial) ----
            # Weight-stationary orientation: out^T[n, i] accumulated per
            # 128-wide n-tile; wout is the lhsT so each weight tile serves 8
            # back-to-back matmuls; output DMAs as [DIM, N] (host transposes).
            with (
                tc.tile_pool(name="po", bufs=2, space="PSUM") as pop,
                tc.tile_pool(name="ob", bufs=3) as obp,
            ):
                for nk in range(8):
                    po = pop.tile([128, 4, 512], f32, tag="po")
                    for ct in range(2):
                        for tc4 in range(4):
                            nc.tensor.matmul(
                                po[:, tc4, :],
                                wout_sb[:, ct, 128 * nk : 128 * nk + 128],
                                ao[:, ct, 512 * tc4 : 512 * tc4 + 512],
                                start=(ct == 0),
                                stop=(ct == 1),
                            )
                    ob = obp.tile([128, 4, 512], f16, tag="ob")
                    # quarter-evictions alternating ACT/DVE: po's banks free
                    # ~0.9us after its last matmul instead of ~2.2us, so the
                    # po rotation never bubbles the PE
                    for q in range(4):
                        if (q + nk) % 2 == 0:
                            nc.scalar.copy(ob[:, q, :], po[:, q, :])
                        else:
                            nc.vector.tensor_copy(ob[:, q, :], po[:, q, :])
                    r = slice(128 * nk, 128 * nk + 128)
                    if nk == 7:
                        # last tile: split the DMA across both queues to
                        # shorten the drain tail
                        nc.sync.dma_start(out=outp_h[r, 0:1024], in_=ob[:, 0:2, :])
                        nc.gpsimd.dma_start(
                            out=outp_h[r, 1024:2048], in_=ob[:, 2:4, :]
                        )
                    else:
                        # alternate DMA queues so transfers run in parallel
                        dq = nc.sync if nk % 2 == 0 else nc.gpsimd
                        dq.dma_start(out=outp_h[r, :], in_=ob)
    nc.finalize()
    return nc


# Per-head d-permutation: SBUF row r (0..63) holds head dim DPERM[r].
# Rows 0-15: evens of pairs t=0..15, 16-31: odds of t=0..15,
# 32-47: evens of t=16..31, 48-63: odds of t=16..31.
DPERM = (
    [2 * t for t in range(16)]
    + [2 * t + 1 for t in range(16)]
    + [2 * t for t in range(16, 32)]
    + [2 * t + 1 for t in range(16, 32)]
)
# freq index for row r: t = r%16 + 16*(r//32)
ROW_T = [r % 16 + 16 * (r // 32) for r in range(64)]
# sign of the sin term for row r: -1 for even-output rows (r%32 < 16)
ROW_SIGN = [-1.0 if (r % 32) < 16 else 1.0 for r in range(64)]


def make_core_inputs(x, Wqkv, Wout, c):
    """Host-side shard prep for core c: batch b=c//4, heads [4*(c%4) .. +4)."""
    b = c // 4
    g = c % 4
    hs = [4 * g + i for i in range(HPC)]
    W4 = np.asarray(Wqkv, np.float32).reshape(DIM, 3, H, DH)
    xt = np.asarray(x, np.float32)[b].T  # [DIM, N]

    # xt packed [p][ch][a][n]
    xt_p = xt.reshape(KT, 128, NCH, 512).transpose(1, 2, 0, 3)  # [128, ch, a, n]
    xt_pack = np.ascontiguousarray(xt_p.reshape(128, NCH * KT * 512))

    # wqk columns: tiles mt=0,1 -> q pairs, mt=2,3 -> k pairs; 64 d-permuted
    # cols per head, head A then head B within each tile.
    cols = []
    for qk in (0, 1):
        for pair in (0, 1):
            for hh in (hs[2 * pair], hs[2 * pair + 1]):
                cols.append(W4[:, qk, hh, :][:, DPERM])
    wqk = np.concatenate(cols, axis=1)  # [DIM, 512]
    wqk_pack = np.ascontiguousarray(
        wqk.reshape(KT, 128, 512).transpose(1, 0, 2).reshape(128, KT * 512)
    )

    wv = W4[:, 2, hs, :].reshape(DIM, 256)
    wv_pack = np.ascontiguousarray(
        wv.reshape(KT, 128, 256).transpose(1, 0, 2).reshape(128, KT * 256)
    )

    wout = np.asarray(Wout, np.float32).reshape(H, DH, DIM)[hs].reshape(256, DIM)
    wout_pack = np.ascontiguousarray(
        wout.reshape(2, 128, DIM).transpose(1, 0, 2).reshape(128, 2 * DIM)
    )

    pos = np.arange(N, dtype=np.float64)
    inv = 1.0 / (ROPE_BASE ** (np.arange(0, DH, 2, dtype=np.float64) / DH))  # [32]
    ang = inv[:, None] * pos[None, :]  # [32, N]
    cos_t = np.cos(ang)  # [32 freqs, N]
    sin_t = np.sin(ang)
    rows_t = np.array(ROW_T * 2)  # 128 rows (two 64-row head halves)
    sign = np.array(ROW_SIGN * 2)[:, None]
    cosb = cos_t[rows_t].astype(np.float32)  # [128, N]
    sinb = (sign * sin_t[rows_t]).astype(np.float32)

    return {
        "xt": xt_pack.astype(F16),
        "wqk": wqk_pack.astype(F16),
        "wv": wv_pack.astype(F16),
        "wout": wout_pack.astype(F16),
        "cosb": cosb.astype(F16),
        "sinb": sinb.astype(F16),
    }


def kernel(x, Wqkv, Wout, _trace=False, _tmpdir=None):
    _concourse()
    from concourse.bass_utils import run_bass_kernel_spmd

    if "nc" not in _prog_cache:
        _prog_cache["nc"] = build_program()
    nc = _prog_cache["nc"]
    in_maps = [make_core_inputs(x, Wqkv, Wout, c) for c in range(NCORES)]
    res = run_bass_kernel_spmd(
        nc, in_maps, list(range(NCORES)), trace=_trace, tmpdir=_tmpdir
    )
    out = np.zeros((B, N, DIM), np.float32)
    for c in range(NCORES):
        out[c // 4] += res.results[c]["outp"].astype(np.float32).T
    if _trace:
        return out, res
    return out
